# revision 2
# baseline (speedup 1.0000x reference)
"""Grouped MLP (MoE expert MLP) kernel for one TRN2 chip (8 NeuronCores).

Expert-parallel: expert e's tokens + weights go to core e (NE == n_cores == 8).
Per core computes out = gelu(x_e @ w1_e.T) @ w2_e with both matmuls on the
TensorEngine in bf16 (fp32 PSUM accumulation).

Layout trick: host passes x_e and w1_e pre-transposed (contraction dim on
partitions) so the device does zero transposes:
  matmul1: hT[f, t] = sum_h w1T[h, f] * xT[h, t]      (lhsT = w1T, rhs = xT)
  gelu    : on PSUM -> SBUF (ScalarE), output bf16
  matmul2: out[t, d] = sum_f hT[f, t] * w2[f, d]      (lhsT = hT, rhs = w2)

Weights stay resident in SBUF (bf16: 8 MB + 8 MB). w1 is streamed in f-major
slices so the first matmul chain only waits on the first slice (~1 us) instead
of the whole 8 MB load (~25 us). Tokens stream in chunks of TC=512 with the
second matmul accumulating over all of FFN in PSUM.
"""

import numpy as np
import ml_dtypes

NE = 8      # experts == cores
HID = 1024
FFN = 4096

_BF16 = ml_dtypes.bfloat16


def _install_axon_profile_hook():
    """Make run_bass_kernel_spmd(trace=True) usable in containers whose antenv
    package lacks axon_hooks. No-op if the real module is importable."""
    try:
        import antenv.axon_hooks  # noqa: F401
        return
    except ImportError:
        pass
    try:
        import sys
        import types

        import antenv  # noqa: F401

        mod = types.ModuleType("antenv.axon_hooks")
        mod._hook = None

        def set_axon_ntff_profile_hook(h):
            mod._hook = h

        def get_axon_ntff_profile_hook():
            return mod._hook

        mod.set_axon_ntff_profile_hook = set_axon_ntff_profile_hook
        mod.get_axon_ntff_profile_hook = get_axon_ntff_profile_hook
        sys.modules["antenv.axon_hooks"] = mod

        from trn_agent_boot.trn_boot import _ntff_profile_via_ctypes

        so_path = "/opt/axon/libaxon_pjrt.so"
        hook = _ntff_profile_via_ctypes(so_path)
        if hook is not None:
            mod._hook = hook
    except Exception:
        pass


def _build(T):
    """Build + compile the per-core Bass kernel for T tokens (multiple of 512)."""
    import concourse.mybir as mybir
    import concourse.tile as tile
    from concourse import bacc

    TC = 512            # token chunk (moving-operand N for matmul1)
    HC = HID // 128     # 8 contraction chunks for matmul1
    FC = FFN // 128     # 32 f chunks
    ND = HID // 512     # 2 output column halves
    NT = T // TC

    # w1 arrives in f-major blocks: small blocks first so the first matmul
    # chain (fi=0) can start ~1 us in, larger blocks after for DMA efficiency.
    W1_BLOCKS = [128] * 4 + [512] * 7
    assert sum(W1_BLOCKS) == FFN

    nc = bacc.Bacc("TRN2", target_bir_lowering=False, debug=False, num_devices=NE)
    xt = nc.declare_dram_parameter("xt", [HID, T], mybir.dt.bfloat16, isOutput=False)
    w1t = nc.declare_dram_parameter("w1t", [HID, FFN], mybir.dt.bfloat16, isOutput=False)
    w2 = nc.declare_dram_parameter("w2", [FFN, HID], mybir.dt.bfloat16, isOutput=False)
    out = nc.declare_dram_parameter("out", [T, HID], mybir.dt.float32, isOutput=True)

    xt_r = xt[:].rearrange("(c p) t -> c p t", p=128)
    w1t_r = w1t[:].rearrange("(c p) f -> c p f", p=128)
    w2_r = w2[:].rearrange("(c p) d -> c p d", p=128)

    with tile.TileContext(nc) as tc:
        with (
            tc.tile_pool(name="weights", bufs=1) as wpool,
            tc.tile_pool(name="xin", bufs=2) as xpool,
            tc.tile_pool(name="hmid", bufs=1) as hpool,
            tc.tile_pool(name="oout", bufs=3) as opool,
            tc.tile_pool(name="ph", bufs=3, space="PSUM") as ph_pool,
            tc.tile_pool(name="po", bufs=3, space="PSUM") as po_pool,
        ):
            w1t_sb = wpool.tile([128, HC, FFN], mybir.dt.bfloat16, tag="w1t")
            w2_sb = wpool.tile([128, FC, HID], mybir.dt.bfloat16, tag="w2")
            f0 = 0
            for blk in W1_BLOCKS:
                for c in range(HC):
                    nc.sync.dma_start(
                        out=w1t_sb[:, c, f0:f0 + blk], in_=w1t_r[c, :, f0:f0 + blk]
                    )
                f0 += blk
            for c in range(FC):
                nc.sync.dma_start(out=w2_sb[:, c, :], in_=w2_r[c])

            for t in range(NT):
                xt_sb = xpool.tile([128, HC, TC], mybir.dt.bfloat16, tag="xt")
                for c in range(HC):
                    nc.sync.dma_start(
                        out=xt_sb[:, c, :], in_=xt_r[c, :, t * TC:(t + 1) * TC]
                    )
                h_sb = hpool.tile([128, FC, TC], mybir.dt.bfloat16, tag="h")
                for fi in range(FC):
                    ph = ph_pool.tile([128, TC], mybir.dt.float32, tag="ph")
                    for c in range(HC):
                        nc.tensor.matmul(
                            ph,
                            w1t_sb[:, c, fi * 128:(fi + 1) * 128],
                            xt_sb[:, c, :],
                            start=(c == 0),
                            stop=(c == HC - 1),
                        )
                    nc.scalar.activation(
                        h_sb[:, fi, :], ph, mybir.ActivationFunctionType.Gelu
                    )
                for ti in range(TC // 128):
                    o_sb = opool.tile([128, HID], mybir.dt.float32, tag="o")
                    for d in range(ND):
                        po = po_pool.tile([128, 512], mybir.dt.float32, tag="po")
                        for fi in range(FC):
                            nc.tensor.matmul(
                                po,
                                h_sb[:, fi, ti * 128:(ti + 1) * 128],
                                w2_sb[:, fi, d * 512:(d + 1) * 512],
                                start=(fi == 0),
                                stop=(fi == FC - 1),
                            )
                        nc.vector.tensor_copy(o_sb[:, d * 512:(d + 1) * 512], po)
                    row0 = t * TC + ti * 128
                    nc.sync.dma_start(out=out[row0:row0 + 128, :], in_=o_sb)

    nc.compile()
    return nc


_compiled = {}

LAST_RESULT = None


def kernel(x, tokens_per_expert, w1, w2):
    from concourse.bass_utils import run_bass_kernel_spmd

    _install_axon_profile_hook()

    x = np.asarray(x)
    w1 = np.asarray(w1)
    w2 = np.asarray(w2)
    tpe = np.asarray(tokens_per_expert).astype(np.int64)
    assert tpe.shape == (NE,)
    bounds = np.concatenate([[0], np.cumsum(tpe)])
    total = int(bounds[-1])
    maxt = max(int(tpe.max()), 1)
    T = ((maxt + 511) // 512) * 512

    if T not in _compiled:
        _compiled[T] = _build(T)
    nc = _compiled[T]

    in_maps = []
    for e in range(NE):
        te = int(tpe[e])
        xe = np.zeros((T, HID), dtype=np.float32)
        xe[:te] = x[bounds[e]:bounds[e + 1]]
        in_maps.append(
            {
                "xt": np.ascontiguousarray(xe.T).astype(_BF16),
                "w1t": np.ascontiguousarray(w1[e].T).astype(_BF16),
                "w2": np.ascontiguousarray(w2[e]).astype(_BF16),
            }
        )

    res = run_bass_kernel_spmd(nc, in_maps, core_ids=list(range(NE)))
    global LAST_RESULT
    LAST_RESULT = res

    out = np.zeros((x.shape[0], HID), dtype=np.float32)
    for e in range(NE):
        te = int(tpe[e])
        out[bounds[e]:bounds[e + 1]] = res.results[e]["out"][:te]
    assert total <= x.shape[0]
    return out


# revision 5
# speedup vs baseline: 1.1503x; 1.1503x over previous
"""Grouped MLP (MoE expert MLP) kernel for one TRN2 chip (8 NeuronCores).

Expert-parallel: expert e's tokens + weights go to core e (NE == n_cores == 8).
Per core computes out = gelu(x_e @ w1_e.T) @ w2_e with both matmuls on the
TensorEngine in bf16 (fp32 PSUM accumulation).

Layout trick: host passes x_e and w1_e pre-transposed (contraction dim on
partitions) so the device does zero transposes:
  matmul1: hT[f, t] = sum_h w1T[h, f] * xT[h, t]      (lhsT = w1T, rhs = xT)
  gelu    : on PSUM -> SBUF (ScalarE), output bf16
  matmul2: out[t, d] = sum_f hT[f, t] * w2[f, d]      (lhsT = hT, rhs = w2)

Weights stay resident in SBUF (bf16: 8 MB + 8 MB). w1 is streamed in f-major
slices so the first matmul chain only waits on the first slice (~1 us) instead
of the whole 8 MB load (~25 us). Tokens stream in chunks of TC=512 with the
second matmul accumulating over all of FFN in PSUM.
"""

import numpy as np
import ml_dtypes

NE = 8      # experts == cores
HID = 1024
FFN = 4096

_BF16 = ml_dtypes.bfloat16


def _install_axon_profile_hook():
    """Make run_bass_kernel_spmd(trace=True) usable in containers whose antenv
    package lacks axon_hooks. No-op if the real module is importable."""
    try:
        import antenv.axon_hooks  # noqa: F401
        return
    except ImportError:
        pass
    try:
        import sys
        import types

        import antenv  # noqa: F401

        mod = types.ModuleType("antenv.axon_hooks")
        mod._hook = None

        def set_axon_ntff_profile_hook(h):
            mod._hook = h

        def get_axon_ntff_profile_hook():
            return mod._hook

        mod.set_axon_ntff_profile_hook = set_axon_ntff_profile_hook
        mod.get_axon_ntff_profile_hook = get_axon_ntff_profile_hook
        sys.modules["antenv.axon_hooks"] = mod

        from trn_agent_boot.trn_boot import _ntff_profile_via_ctypes

        so_path = "/opt/axon/libaxon_pjrt.so"
        hook = _ntff_profile_via_ctypes(so_path)
        if hook is not None:
            mod._hook = hook
    except Exception:
        pass


def _build(T):
    """Build + compile the per-core Bass kernel for T tokens (multiple of 512)."""
    import concourse.mybir as mybir
    import concourse.tile as tile
    from concourse import bacc

    TC = 512            # token chunk (moving-operand N for matmul1)
    HC = HID // 128     # 8 contraction chunks for matmul1
    FC = FFN // 128     # 32 f chunks
    ND = HID // 512     # 2 output column halves
    NT = T // TC

    # w1 arrives in f-major blocks: small blocks first so the first matmul
    # chain (fi=0) can start a few us in, larger blocks after for DMA
    # efficiency (DMA line = block * 2B, want >= 2KB steady state).
    W1_BLOCKS = [256, 256, 512, 1024, 1024, 1024]
    assert sum(W1_BLOCKS) == FFN

    nc = bacc.Bacc("TRN2", target_bir_lowering=False, debug=False, num_devices=NE)
    xt = nc.declare_dram_parameter("xt", [HID, T], mybir.dt.bfloat16, isOutput=False)
    w1t = nc.declare_dram_parameter("w1t", [HID, FFN], mybir.dt.bfloat16, isOutput=False)
    w2 = nc.declare_dram_parameter("w2", [FFN, HID], mybir.dt.bfloat16, isOutput=False)
    out = nc.declare_dram_parameter("out", [T, HID], mybir.dt.float32, isOutput=True)

    xt_r = xt[:].rearrange("(c p) t -> c p t", p=128)
    w1t_r = w1t[:].rearrange("(c p) f -> c p f", p=128)
    w2_r = w2[:].rearrange("(c p) d -> c p d", p=128)

    with tile.TileContext(nc) as tc:
        with (
            tc.tile_pool(name="weights", bufs=1) as wpool,
            tc.tile_pool(name="xin", bufs=2) as xpool,
            tc.tile_pool(name="hmid", bufs=1) as hpool,
            tc.tile_pool(name="oout", bufs=3) as opool,
            tc.tile_pool(name="ph", bufs=3, space="PSUM") as ph_pool,
            tc.tile_pool(name="po", bufs=3, space="PSUM") as po_pool,
        ):
            w1t_sb = wpool.tile([128, HC, FFN], mybir.dt.bfloat16, tag="w1t")
            w2_sb = wpool.tile([128, FC, HID], mybir.dt.bfloat16, tag="w2")

            # DMA-queue completion is in-order, so issue order = arrival
            # order: first token chunk, then w1 (f-major), then w2.  The
            # first matmul chain then only waits ~x0 + w1 block 0.
            x_tiles = [None] * NT
            x_tiles[0] = xpool.tile([128, HC, TC], mybir.dt.bfloat16, tag="xt", name="xt0")
            for c in range(HC):
                nc.sync.dma_start(out=x_tiles[0][:, c, :], in_=xt_r[c, :, 0:TC])
            f0 = 0
            for blk in W1_BLOCKS:
                for c in range(HC):
                    nc.sync.dma_start(
                        out=w1t_sb[:, c, f0:f0 + blk], in_=w1t_r[c, :, f0:f0 + blk]
                    )
                f0 += blk
            for c in range(FC):
                nc.sync.dma_start(out=w2_sb[:, c, :], in_=w2_r[c])

            for t in range(NT):
                if x_tiles[t] is None:
                    x_tiles[t] = xpool.tile(
                        [128, HC, TC], mybir.dt.bfloat16, tag="xt", name=f"xt{t}"
                    )
                    for c in range(HC):
                        nc.sync.dma_start(
                            out=x_tiles[t][:, c, :],
                            in_=xt_r[c, :, t * TC:(t + 1) * TC],
                        )
                xt_sb = x_tiles[t]
                h_sb = hpool.tile([128, FC, TC], mybir.dt.bfloat16, tag="h")
                for fi in range(FC):
                    ph = ph_pool.tile([128, TC], mybir.dt.float32, tag="ph")
                    for c in range(HC):
                        nc.tensor.matmul(
                            ph,
                            w1t_sb[:, c, fi * 128:(fi + 1) * 128],
                            xt_sb[:, c, :],
                            start=(c == 0),
                            stop=(c == HC - 1),
                        )
                    nc.scalar.activation(
                        h_sb[:, fi, :], ph, mybir.ActivationFunctionType.Gelu
                    )
                for ti in range(TC // 128):
                    o_sb = opool.tile([128, HID], mybir.dt.float32, tag="o")
                    for d in range(ND):
                        po = po_pool.tile([128, 512], mybir.dt.float32, tag="po")
                        for fi in range(FC):
                            nc.tensor.matmul(
                                po,
                                h_sb[:, fi, ti * 128:(ti + 1) * 128],
                                w2_sb[:, fi, d * 512:(d + 1) * 512],
                                start=(fi == 0),
                                stop=(fi == FC - 1),
                            )
                        nc.vector.tensor_copy(o_sb[:, d * 512:(d + 1) * 512], po)
                    row0 = t * TC + ti * 128
                    nc.sync.dma_start(out=out[row0:row0 + 128, :], in_=o_sb)

    nc.compile()
    return nc


_compiled = {}

LAST_RESULT = None


def kernel(x, tokens_per_expert, w1, w2):
    from concourse.bass_utils import run_bass_kernel_spmd

    _install_axon_profile_hook()

    x = np.asarray(x)
    w1 = np.asarray(w1)
    w2 = np.asarray(w2)
    tpe = np.asarray(tokens_per_expert).astype(np.int64)
    assert tpe.shape == (NE,)
    bounds = np.concatenate([[0], np.cumsum(tpe)])
    total = int(bounds[-1])
    maxt = max(int(tpe.max()), 1)
    T = ((maxt + 511) // 512) * 512

    if T not in _compiled:
        _compiled[T] = _build(T)
    nc = _compiled[T]

    in_maps = []
    for e in range(NE):
        te = int(tpe[e])
        xe = np.zeros((T, HID), dtype=np.float32)
        xe[:te] = x[bounds[e]:bounds[e + 1]]
        in_maps.append(
            {
                "xt": np.ascontiguousarray(xe.T).astype(_BF16),
                "w1t": np.ascontiguousarray(w1[e].T).astype(_BF16),
                "w2": np.ascontiguousarray(w2[e]).astype(_BF16),
            }
        )

    res = run_bass_kernel_spmd(nc, in_maps, core_ids=list(range(NE)))
    global LAST_RESULT
    LAST_RESULT = res

    out = np.zeros((x.shape[0], HID), dtype=np.float32)
    for e in range(NE):
        te = int(tpe[e])
        out[bounds[e]:bounds[e + 1]] = res.results[e]["out"][:te]
    assert total <= x.shape[0]
    return out
